# revision 9
# baseline (speedup 1.0000x reference)
"""Trainium2 Bass kernel for nn_CrossPatchModule.

Math (validated against the reference):
  The reference unfolds x[b,c] (512x512) into an 8x8 grid of 64x64 blocks
  (block index p = pi*8 + pj), adds pos[c, q] to block q, cyclically
  shifts blocks per channel, and folds back:

      out[b, c, block p] = x[b, c, block q] + pos[c, q],   q = (p + c) % 64

  where pos = abs_pos[0, 0, :, :, 0, 0]  (shape [64, 64], [channel, block]).

Strategy:
  - Pure data-parallel: 8 batch samples -> 8 NeuronCores (one sample each).
  - Per core, loop over 32 channel pairs. SBUF tile layout per pair:
      T[c2*64 + a, qi*512 + qj*64 + d] = x[c, qi*64 + a, qj*64 + d]
    i.e. partition = (channel-of-pair, row-within-block), free dim = blocks
    in raster order x columns. Innermost contiguous DMA run = 2048 B (a full
    image row chunk), 128 partitions -> full DMA efficiency.
  - In this layout the block shift is a cyclic rotation of the free dim by
    64*c elements, so shift+bias fuse into two tensor_tensor adds per
    channel that read the input tile at shifted free offsets.
  - The per-(channel, block) bias (constant along partitions) is broadcast
    across partitions with a K=1 TensorE outer product (masks ^T @ bias_row)
    into PSUM; VectorE reads it as the second tensor_tensor operand.
"""

import os
import numpy as np

import concourse.bacc as bacc
import concourse.mybir as mybir
from concourse.tile import TileContext
from concourse.bass_utils import run_bass_kernel_spmd

B, C, H, W = 8, 64, 512, 512
PN = 64          # number of 64x64 blocks per image (8x8 grid) == C
KW = 64          # block width
FD = PN * KW     # free dim of a channel slice: 64 blocks x 64 cols = 4096
F32 = mybir.dt.float32

LAST_RESULTS = None  # BassKernelResults of the most recent run (for test.py)

_NC_CACHE = {}


def _build_nc():
    nc = bacc.Bacc("TRN2")

    x = nc.dram_tensor("x", [C, H, W], F32, kind="ExternalInput")
    # bias[i] = concat(bias_p[2i], bias_p[2i+1]) — both channels of pair i
    biasd = nc.dram_tensor("bias", [C // 2, 2 * FD], F32, kind="ExternalInput")
    # masks[0, 0:128] selects partitions 0-63; masks[0, 128:256] selects 64-127
    masksd = nc.dram_tensor("masks", [1, 256], F32, kind="ExternalInput")
    out = nc.dram_tensor("out", [C, H, W], F32, kind="ExternalOutput")

    # [pair, c2, a, qi, (qj d)]: flattened order (c2, a) ~ 128 partitions,
    # (qi, qj*64+d) ~ 4096 free elems; innermost contiguous run = 512 f32.
    xv = x.rearrange("(cp c2) (qi a) (qj d) -> cp c2 a qi (qj d)", c2=2, a=64, d=64)
    ov = out.rearrange("(cp c2) (pi a) (pj d) -> cp c2 a pi (pj d)", c2=2, a=64, d=64)

    with TileContext(nc) as tc:
        with (
            tc.tile_pool(name="const", bufs=1) as cpool,
            tc.tile_pool(name="io", bufs=3) as iopool,
            tc.tile_pool(name="ps", bufs=1, space="PSUM") as pspool,
        ):
            mask_sb = cpool.tile([1, 256], F32, tag="masks")
            nc.sync.dma_start(out=mask_sb[:], in_=masksd[:])

            for i in range(C // 2):
                c0 = 2 * i
                tin = iopool.tile([128, FD], F32, tag="tin")
                # one DMA per channel half: DMA APs are limited to 3 dims
                nc.sync.dma_start(out=tin[0:64, :], in_=xv[i, 0])
                nc.sync.dma_start(out=tin[64:128, :], in_=xv[i, 1])

                # stage pair bias rows into partition 0 (PE operands need base
                # partition 0/32/64)
                stage = iopool.tile([1, 2 * FD], F32, tag="stage")
                nc.sync.dma_start(out=stage[:], in_=biasd[i : i + 1, :])

                # bias broadcast: psum[0:64, f] = bias[c0, f]; psum[64:128, f] = bias[c0+1, f]
                ps = pspool.tile([128, FD], F32, tag="ps")
                for n in range(FD // 512):
                    sl = slice(n * 512, (n + 1) * 512)
                    nc.tensor.matmul(
                        ps[:, sl], mask_sb[0:1, 0:128], stage[0:1, sl],
                        start=True, stop=False,
                    )
                    nc.tensor.matmul(
                        ps[:, sl], mask_sb[0:1, 128:256],
                        stage[0:1, FD + n * 512 : FD + (n + 1) * 512],
                        start=False, stop=True,
                    )

                tout = iopool.tile([128, FD], F32, tag="tout")
                for c2 in range(2):
                    c = c0 + c2
                    rows = slice(c2 * 64, c2 * 64 + 64)
                    shift = c * KW          # free-dim rotation amount
                    split = FD - shift      # out[f < split] <- in[f + shift]
                    nc.vector.tensor_add(
                        out=tout[rows, 0:split],
                        in0=tin[rows, shift:FD],
                        in1=ps[rows, 0:split],
                    )
                    if shift:
                        nc.vector.tensor_add(
                            out=tout[rows, split:FD],
                            in0=tin[rows, 0:shift],
                            in1=ps[rows, split:FD],
                        )

                nc.sync.dma_start(out=ov[i, 0], in_=tout[0:64, :])
                nc.sync.dma_start(out=ov[i, 1], in_=tout[64:128, :])

    nc.finalize()
    return nc


def _host_bias(abs_pos: np.ndarray) -> np.ndarray:
    pos = np.asarray(abs_pos, dtype=np.float32)[0, 0, :, :, 0, 0]  # [C, PN]
    idx = (np.arange(PN)[None, :] + np.arange(C)[:, None]) % PN    # [C, p] -> q
    bias = np.take_along_axis(pos, idx, axis=1)                    # [C, PN]
    bias = np.repeat(bias, KW, axis=1)                             # [C, 4096]
    return np.ascontiguousarray(bias.reshape(C // 2, 2 * FD))      # [32, 8192]


def kernel(x: np.ndarray, abs_pos: np.ndarray) -> np.ndarray:
    global LAST_RESULTS
    x = np.ascontiguousarray(np.asarray(x, dtype=np.float32))
    assert x.shape == (B, C, H, W), x.shape

    bias = _host_bias(abs_pos)
    masks = np.zeros((1, 256), np.float32)
    masks[0, 0:64] = 1.0
    masks[0, 192:256] = 1.0

    if "nc" not in _NC_CACHE:
        _NC_CACHE["nc"] = _build_nc()
    nc = _NC_CACHE["nc"]

    in_maps = [{"x": x[b], "bias": bias, "masks": masks} for b in range(B)]
    res = run_bass_kernel_spmd(
        nc,
        in_maps,
        core_ids=list(range(B)),
        trace=bool(os.environ.get("KERNEL_TRACE")),
    )
    LAST_RESULTS = res
    return np.stack([res.results[b]["out"] for b in range(B)], axis=0)


# revision 15
# speedup vs baseline: 1.7067x; 1.7067x over previous
"""Trainium2 Bass kernel for nn_CrossPatchModule.

Math (validated against the reference):
  The reference unfolds x[b,c] (512x512) into an 8x8 grid of 64x64 blocks
  (block index p = pi*8 + pj), adds pos[c, q] to block q, cyclically
  shifts blocks per channel, and folds back:

      out[b, c, block p] = x[b, c, block q] + pos[c, q],   q = (p + c) % 64

  where pos = abs_pos[0, 0, :, :, 0, 0]  (shape [64, 64], [channel, block]).

Strategy:
  - Pure data-parallel: 8 batch samples -> 8 NeuronCores (one sample each).
  - Per core, loop over 32 channel pairs. SBUF tile layout per pair:
      T[c2*64 + a, qi*512 + qj*64 + d] = x[c, qi*64 + a, qj*64 + d]
    i.e. partition = (channel-of-pair, row-within-block), free dim = blocks
    in raster order x columns. Innermost contiguous DMA run = 2048 B.
  - In this layout the block shift is a cyclic rotation of the free dim by
    64*c elements, so shift+bias fuse into two tensor_tensor adds per
    channel that read the input tile at shifted free offsets.
  - The per-(channel, block) bias lives in SBUF compactly (one scalar per
    block: [128, 32*64] = 1 MiB, replicated across the 64 row-partitions
    host-side) and the DVE add reads it through a stride-0 innermost free
    dim (broadcast_to), so no on-chip broadcast pass is needed.
"""

import os
import numpy as np

import concourse.bacc as bacc
import concourse.mybir as mybir
from concourse.tile import TileContext
from concourse.bass_utils import run_bass_kernel_spmd

B, C, H, W = 8, 64, 512, 512
PN = 64          # number of 64x64 blocks per image (8x8 grid) == C
KW = 64          # block width
FD = PN * KW     # free dim of a channel slice: 64 blocks x 64 cols = 4096
NPAIR = C // 2   # 32 channel pairs
F32 = mybir.dt.float32

LAST_RESULTS = None  # BassKernelResults of the most recent run (for test.py)

_NC_CACHE = {}


def _build_nc():
    nc = bacc.Bacc("TRN2")

    x = nc.dram_tensor("x", [C, H, W], F32, kind="ExternalInput")
    # compact per-block bias, p-ordered:
    #   biasd[part, i*64 + p] = pos[c, (p + c) % 64],  c = 2i + (part >= 64)
    biasd = nc.dram_tensor("bias", [128, NPAIR * PN], F32, kind="ExternalInput")
    out = nc.dram_tensor("out", [C, H, W], F32, kind="ExternalOutput")

    # [pair, c2, a, qi, (qj d)]: flattened order (c2, a) ~ 128 partitions,
    # (qi, qj*64+d) ~ 4096 free elems; innermost contiguous run = 512 f32.
    xv = x.rearrange("(cp c2) (qi a) (qj d) -> cp c2 a qi (qj d)", c2=2, a=64, d=64)
    ov = out.rearrange("(cp c2) (pi a) (pj d) -> cp c2 a pi (pj d)", c2=2, a=64, d=64)

    with TileContext(nc) as tc:
        with (
            tc.tile_pool(name="const", bufs=1) as cpool,
            tc.tile_pool(name="io", bufs=4) as iopool,
        ):
            bias_sb = cpool.tile([128, NPAIR * PN], F32, tag="bias")
            nc.sync.dma_start(out=bias_sb[:], in_=biasd[:])

            for i in range(NPAIR):
                c0 = 2 * i
                tin = iopool.tile([128, FD], F32, tag="tin")
                # one DMA per channel half: DMA APs are limited to 3 dims
                nc.sync.dma_start(out=tin[0:64, :], in_=xv[i, 0])
                nc.sync.dma_start(out=tin[64:128, :], in_=xv[i, 1])

                tout = iopool.tile([128, FD], F32, tag="tout")
                for c2 in range(2):
                    c = c0 + c2
                    rows = slice(c2 * 64, c2 * 64 + 64)
                    shift = c * KW          # free-dim rotation amount
                    split = FD - shift      # out[f < split] <- in[f + shift]
                    nblk = PN - c           # blocks in the first segment
                    nc.vector.tensor_add(
                        out=tout[rows, 0:split].rearrange(
                            "r (n d) -> r n d", d=KW
                        ),
                        in0=tin[rows, shift:FD].rearrange("r (n d) -> r n d", d=KW),
                        in1=bias_sb[rows, i * PN : i * PN + nblk][
                            :, :, None
                        ].broadcast_to([64, nblk, KW]),
                    )
                    if shift:
                        nc.vector.tensor_add(
                            out=tout[rows, split:FD].rearrange(
                                "r (n d) -> r n d", d=KW
                            ),
                            in0=tin[rows, 0:shift].rearrange(
                                "r (n d) -> r n d", d=KW
                            ),
                            in1=bias_sb[rows, i * PN + nblk : (i + 1) * PN][
                                :, :, None
                            ].broadcast_to([64, c, KW]),
                        )

                nc.sync.dma_start(out=ov[i, 0], in_=tout[0:64, :])
                nc.sync.dma_start(out=ov[i, 1], in_=tout[64:128, :])

    nc.finalize()
    return nc


def _host_bias(abs_pos: np.ndarray) -> np.ndarray:
    pos = np.asarray(abs_pos, dtype=np.float32)[0, 0, :, :, 0, 0]  # [C, PN]
    idx = (np.arange(PN)[None, :] + np.arange(C)[:, None]) % PN    # [C, p] -> q
    bias = np.take_along_axis(pos, idx, axis=1)                    # [C, PN]
    bias = bias.reshape(NPAIR, 2, PN).transpose(1, 0, 2)           # [2, NPAIR, PN]
    bias = np.repeat(bias, 64, axis=0)                             # [128, NPAIR, PN]
    return np.ascontiguousarray(bias.reshape(128, NPAIR * PN))


def kernel(x: np.ndarray, abs_pos: np.ndarray) -> np.ndarray:
    global LAST_RESULTS
    x = np.ascontiguousarray(np.asarray(x, dtype=np.float32))
    assert x.shape == (B, C, H, W), x.shape

    bias = _host_bias(abs_pos)

    if "nc" not in _NC_CACHE:
        _NC_CACHE["nc"] = _build_nc()
    nc = _NC_CACHE["nc"]

    in_maps = [{"x": x[b], "bias": bias} for b in range(B)]
    res = run_bass_kernel_spmd(
        nc,
        in_maps,
        core_ids=list(range(B)),
        trace=bool(os.environ.get("KERNEL_TRACE")),
    )
    LAST_RESULTS = res
    return np.stack([res.results[b]["out"] for b in range(B)], axis=0)


# revision 17
# speedup vs baseline: 2.6581x; 1.5574x over previous
"""Trainium2 Bass kernel for nn_CrossPatchModule.

Math (validated against the reference):
  The reference unfolds x[b,c] (512x512) into an 8x8 grid of 64x64 blocks
  (block index p = pi*8 + pj), adds pos[c, q] to block q, cyclically
  shifts blocks per channel, and folds back:

      out[b, c, block p] = x[b, c, block q] + pos[c, q],   q = (p + c) % 64

  where pos = abs_pos[0, 0, :, :, 0, 0]  (shape [64, 64], [channel, block]).

Strategy:
  - Pure data-parallel: 8 batch samples -> 8 NeuronCores (one sample each).
  - Per core, loop over 32 channel pairs. SBUF tile layout per pair:
      T[c2*64 + a, qi*512 + qj*64 + d] = x[c, qi*64 + a, qj*64 + d]
    i.e. partition = (channel-of-pair, row-within-block), free dim = blocks
    in raster order x columns. Innermost contiguous DMA run = 2048 B.
  - In this layout the block shift is a cyclic rotation of the free dim by
    64*c elements, so shift+bias fuse into two tensor_tensor adds per
    channel that read the input tile at shifted free offsets.
  - The per-(channel, block) bias lives in SBUF compactly (one scalar per
    block: [128, 32*64] = 1 MiB, replicated across the 64 row-partitions
    host-side) and the DVE add reads it through a stride-0 innermost free
    dim (broadcast_to), so no on-chip broadcast pass is needed.
"""

import os
import numpy as np

import concourse.bacc as bacc
import concourse.mybir as mybir
from concourse.tile import TileContext
from concourse.bass_utils import run_bass_kernel_spmd

B, C, H, W = 8, 64, 512, 512
PN = 64          # number of 64x64 blocks per image (8x8 grid) == C
KW = 64          # block width
FD = PN * KW     # free dim of a channel slice: 64 blocks x 64 cols = 4096
NPAIR = C // 2   # 32 channel pairs
F32 = mybir.dt.float32

LAST_RESULTS = None  # BassKernelResults of the most recent run (for test.py)

_NC_CACHE = {}


def _build_nc():
    nc = bacc.Bacc("TRN2")

    x = nc.dram_tensor("x", [C, H, W], F32, kind="ExternalInput")
    # compact per-block bias, p-ordered:
    #   biasd[part, i*64 + p] = pos[c, (p + c) % 64],  c = 2i + (part >= 64)
    biasd = nc.dram_tensor("bias", [128, NPAIR * PN], F32, kind="ExternalInput")
    out = nc.dram_tensor("out", [C, H, W], F32, kind="ExternalOutput")

    # [pair, c2, a, qi, (qj d)]: flattened order (c2, a) ~ 128 partitions,
    # (qi, qj*64+d) ~ 4096 free elems; innermost contiguous run = 512 f32.
    xv = x.rearrange("(cp c2) (qi a) (qj d) -> cp c2 a qi (qj d)", c2=2, a=64, d=64)
    ov = out.rearrange("(cp c2) (pi a) (pj d) -> cp c2 a pi (pj d)", c2=2, a=64, d=64)

    with TileContext(nc) as tc:
        with (
            tc.tile_pool(name="const", bufs=1) as cpool,
            tc.tile_pool(name="io", bufs=4) as iopool,
        ):
            bias_sb = cpool.tile([128, NPAIR * PN], F32, tag="bias")
            nc.sync.dma_start(out=bias_sb[:], in_=biasd[:])

            for i in range(NPAIR):
                c0 = 2 * i
                tin = iopool.tile([128, FD], F32, tag="tin")
                # one DMA per channel half (DMA APs are limited to 3 dims),
                # split across the two HWDGE rings (SP + ACT) so the
                # complementary 64-partition halves transfer concurrently
                nc.sync.dma_start(out=tin[0:64, :], in_=xv[i, 0])
                nc.scalar.dma_start(out=tin[64:128, :], in_=xv[i, 1])

                tout = iopool.tile([128, FD], F32, tag="tout")
                for c2 in range(2):
                    c = c0 + c2
                    rows = slice(c2 * 64, c2 * 64 + 64)
                    shift = c * KW          # free-dim rotation amount
                    split = FD - shift      # out[f < split] <- in[f + shift]
                    nblk = PN - c           # blocks in the first segment
                    nc.vector.tensor_add(
                        out=tout[rows, 0:split].rearrange(
                            "r (n d) -> r n d", d=KW
                        ),
                        in0=tin[rows, shift:FD].rearrange("r (n d) -> r n d", d=KW),
                        in1=bias_sb[rows, i * PN : i * PN + nblk][
                            :, :, None
                        ].broadcast_to([64, nblk, KW]),
                    )
                    if shift:
                        nc.vector.tensor_add(
                            out=tout[rows, split:FD].rearrange(
                                "r (n d) -> r n d", d=KW
                            ),
                            in0=tin[rows, 0:shift].rearrange(
                                "r (n d) -> r n d", d=KW
                            ),
                            in1=bias_sb[rows, i * PN + nblk : (i + 1) * PN][
                                :, :, None
                            ].broadcast_to([64, c, KW]),
                        )

                nc.scalar.dma_start(out=ov[i, 0], in_=tout[0:64, :])
                nc.sync.dma_start(out=ov[i, 1], in_=tout[64:128, :])

    nc.finalize()
    return nc


def _host_bias(abs_pos: np.ndarray) -> np.ndarray:
    pos = np.asarray(abs_pos, dtype=np.float32)[0, 0, :, :, 0, 0]  # [C, PN]
    idx = (np.arange(PN)[None, :] + np.arange(C)[:, None]) % PN    # [C, p] -> q
    bias = np.take_along_axis(pos, idx, axis=1)                    # [C, PN]
    bias = bias.reshape(NPAIR, 2, PN).transpose(1, 0, 2)           # [2, NPAIR, PN]
    bias = np.repeat(bias, 64, axis=0)                             # [128, NPAIR, PN]
    return np.ascontiguousarray(bias.reshape(128, NPAIR * PN))


def kernel(x: np.ndarray, abs_pos: np.ndarray) -> np.ndarray:
    global LAST_RESULTS
    x = np.ascontiguousarray(np.asarray(x, dtype=np.float32))
    assert x.shape == (B, C, H, W), x.shape

    bias = _host_bias(abs_pos)

    if "nc" not in _NC_CACHE:
        _NC_CACHE["nc"] = _build_nc()
    nc = _NC_CACHE["nc"]

    in_maps = [{"x": x[b], "bias": bias} for b in range(B)]
    res = run_bass_kernel_spmd(
        nc,
        in_maps,
        core_ids=list(range(B)),
        trace=bool(os.environ.get("KERNEL_TRACE")),
    )
    LAST_RESULTS = res
    return np.stack([res.results[b]["out"] for b in range(B)], axis=0)
